# revision 1
# baseline (speedup 1.0000x reference)
"""Trainium2 Bass kernel for the DMP-rollout Net (nn_Net_60567628808344).

Math
----
The reference integrates, per row r of p = (x*scale).reshape(-1, 27):
    y0 = p[:,0], goal = p[:,1], w = p[:,2:]
    cx_j = (1 - A_X*DT/TAU)^j                     (data independent)
    psi_j = exp(-0.5 (cx_j - c)^2 / sigma2)       (data independent)
    state update is LINEAR:  s_j = M s_{j-1} + [0; k*(Az*Bz*goal + F_j)]
    with F_j = (w @ psi_j) * cx_j * (goal-y0) / sum(psi_j)
So the whole 301-step rollout collapses to a closed form
    out[r, i] = A[i]*y0_r + B[i]*goal_r + (goal_r - y0_r) * (w_r @ H[i, :])
with constant A,B (301,), H (301, 25) precomputed in float64 on host.

Device work per core (8-way batch data-parallel) is then a single
(16384 x 27) @ (27 x 301) matmul + the 19.7 MB/core output writeback,
which is HBM-write roofline bound.

Layout: per-core rows are packed 4 tiles at a time into the partition
dim (4 groups of 32 partitions, K padded 27->32 with zeros) so each
quad issues 4 row-group-packed concurrent matmuls (tile_position
(32q, 0) auto-derived from base partitions).
"""

import numpy as np

# DMP hyperparameters fixed by Net.__init__ (hardcoded per problem spec)
N = 25
DOF = 2
DT = 0.01
TAU = 3.0
A_X = 2.0
A_Z = 48.0
B_Z = A_Z / 4.0
T = 301                    # time steps
BATCH = 65536
PARAM_DIM = DOF * (N + 2)  # 54
NCORES = 8

ROWS = BATCH * DOF         # 131072 (B*DOF rows)
RPC = ROWS // NCORES       # 16384 rows per core
TILES = RPC // 128         # 128 tiles of 128 rows per core
QUAD_COLS = RPC // 4       # 4096: vt free dim (4 tiles packed in partition dim)
G = 8                      # tiles per output staging block
NBLK = TILES // G          # 16 output blocks per core
# Blocks per output DMA: small head (start writeback early) and small tail
# (shrink the flush after the last copy). Input DMA also uses a HWDGE lane,
# so at most 7 output DMAs (8 sem lanes total).
DMA_GROUPS = (1, 2, 2, 3, 3, 3, 2)
assert sum(DMA_GROUPS) == NBLK and len(DMA_GROUPS) + 1 <= 8

# float32r (1 cyc/row) fails walrus's matmul ISA check in this build for
# every variant probed (K=128/K=32, with/without tile_position) — stay fp32.
USE_F32R = False

A_QUADS = 8                # quads in the head input chunk (HWDGE)


# ----------------------------------------------------------------------------
# Host-side constant build (exact, float64)
# ----------------------------------------------------------------------------
_const_cache = {}


def _build_constants(c=None, sigma2=None):
    """Return hc (128, 301) float32: rows 32q+p hold
    p==0 -> A, p==1 -> B, p==2+n -> H[:, n], rows 27..31 of each group zero."""
    if c is None:
        c = np.exp(-A_X * np.linspace(0.0, 1.0, N))
    if sigma2 is None:
        sigma2 = (N ** 1.5) / c / A_X
    c = np.asarray(c, np.float64)
    sigma2 = np.asarray(sigma2, np.float64)
    key = (c.tobytes(), sigma2.tobytes())
    if key in _const_cache:
        return _const_cache[key]

    k = DT / TAU
    M = np.array([[1.0, k], [-A_Z * B_Z * k, 1.0 - A_Z * k]])
    P = np.zeros(T + 1)
    Q = np.zeros(T + 1)
    Mn = np.eye(2)
    for n in range(T + 1):
        P[n] = Mn[0, 0]
        Q[n] = Mn[0, 1]
        Mn = Mn @ M

    decay = 1.0 - A_X * DT / TAU
    cx = decay ** np.arange(1, T + 1)                        # cx_1..cx_T
    psi = np.exp(-0.5 * (cx[:, None] - c[None, :]) ** 2 / sigma2[None, :])
    g = psi * (cx / psi.sum(1))[:, None]                     # (T, N)

    A = P[1:T + 1]
    B = k * A_Z * B_Z * np.cumsum(Q[0:T])
    # H[i] = k * sum_{m<=i} Q[i-m] g[m]  -- lower-triangular Toeplitz matvec
    ii = np.arange(T)[:, None]
    mm = np.arange(T)[None, :]
    L = np.where(ii >= mm, Q[np.clip(ii - mm, 0, T)], 0.0)   # (T, T)
    H = k * (L @ g)                                          # (T, N)

    hfull = np.zeros((32, T), np.float32)
    hfull[0] = A.astype(np.float32)
    hfull[1] = B.astype(np.float32)
    hfull[2:2 + N] = H.T.astype(np.float32)
    hc = np.tile(hfull, (4, 1))                              # (128, T)
    _const_cache[key] = hc
    return hc


def _pack_inputs(x, c, sigma2, scale):
    """Build per-core vt arrays (128, 4096) + shared hc (128, 301)."""
    x = np.asarray(x, np.float32)
    if scale is None:
        scale = np.ones(PARAM_DIM, np.float32)
    p = (x * np.asarray(scale, np.float32)).reshape(ROWS, N + 2)
    y0 = p[:, 0]
    goal = p[:, 1]
    u = goal - y0
    v = np.empty((ROWS, N + 2), np.float32)
    v[:, 0] = y0
    v[:, 1] = goal
    v[:, 2:] = p[:, 2:] * u[:, None]

    hc = _build_constants(c, sigma2)

    vts = []
    for i in range(NCORES):
        vc = v[RPC * i:RPC * (i + 1)]                 # (16384, 27)
        # Tile t, lhsT free index f computes local row 128*f + t, so each
        # out-partition owns a contiguous run of HBM rows (linear writeback).
        # row = 128*f + 4*j + q  ->  [f, j, q, p] -> [q, p, j, f], pad p 27->32
        v4 = vc.reshape(128, TILES // 4, 4, N + 2).transpose(2, 3, 1, 0)
        vp = np.zeros((4, 32, TILES // 4, 128), np.float32)
        vp[:, :N + 2] = v4
        vts.append(np.ascontiguousarray(vp.reshape(128, QUAD_COLS)))
    return vts, hc


# ----------------------------------------------------------------------------
# Bass kernel
# ----------------------------------------------------------------------------
_nc_cache = []


def _build_bass():
    if _nc_cache:
        return _nc_cache[0]
    import concourse.bass as bass
    import concourse.mybir as mybir
    from concourse import tile
    import bass_rust
    from concourse.vector_clock import ScopedClock

    class SplitDrainTileContext(tile.TileContext):
        """This walrus build allows a single sync wait per instruction, but
        TileContext's kernel-tail drain carries one wait per live sem lane.
        Split the extras onto standalone single-wait SP nops (same stream, so
        all waits still complete before the barrier + sem clearing)."""

        def _drain_and_barrier(self, tick_clock, wait_clock):
            nc = self.nc
            drain_inst = nc.sync.drain()
            wait_clock.add_sem_waits(
                drain_inst.ins, ScopedClock({None: tick_clock.global_clock})
            )
            si = drain_inst.ins.sync_info
            waits = list(si.on_wait) if si is not None else []
            if len(waits) > 1:
                drain_inst.ins.sync_info = bass_rust.SyncInfo(
                    on_wait=[waits[0]], on_update=list(si.on_update)
                )
                for w in waits[1:]:
                    n = nc.sync.nop(nofuse=True)
                    n.ins.sync_info = bass_rust.SyncInfo(
                        on_wait=[w], on_update=[]
                    )
            nc.all_engine_barrier()
            assert self.sems is not None
            popped = nc._tile_sem_poison_stack.pop()
            assert popped is self._sem_poison
            nc.clear_and_free_semaphores(list(self.sems.allocated().values()))
            nc.all_engine_barrier()

    f32 = mybir.dt.float32
    fmm = mybir.dt.float32r if USE_F32R else f32
    nc = bass.Bass()
    # Input split: a small head chunk (first A_QUADS quads + the 301 constant
    # columns) on HWDGE so compute starts after ~2us, the rest on SWDGE in
    # parallel. Single tensors per chunk keep every matmul at one sync wait
    # (walrus allows a single S3_LW wait slot per self-loading matmul).
    va_d = nc.dram_tensor("va", [128, 128 * A_QUADS + T], fmm, kind="ExternalInput")
    vb_d = nc.dram_tensor(
        "vb", [128, QUAD_COLS - 128 * A_QUADS], fmm, kind="ExternalInput"
    )
    out_d = nc.dram_tensor("out", [RPC, T], f32, kind="ExternalOutput")

    with SplitDrainTileContext(nc) as tc:
        with (
            tc.tile_pool(name="vtp", bufs=1) as vtp,
            tc.tile_pool(name="stage", bufs=1) as stagep,
            tc.tile_pool(name="psum", bufs=7, space="PSUM") as psump,
            tc.tile_pool(name="clm", bufs=1, space="PSUM") as clmp,
        ):
            vtsA = vtp.tile([128, 128 * A_QUADS + T], fmm, tag="vtsA")
            vtsB = vtp.tile([128, QUAD_COLS - 128 * A_QUADS], fmm, tag="vtsB")
            nc.sync.dma_start(vtsA[:], va_d[:])
            nc.gpsimd.dma_start(vtsB[:], vb_d[:])
            hrep = vtsA[:, 128 * A_QUADS:128 * A_QUADS + T]

            def lhsT(j, q):
                if j < A_QUADS:
                    return vtsA[32 * q:32 * q + 32, 128 * j:128 * (j + 1)]
                jb = j - A_QUADS
                return vtsB[32 * q:32 * q + 32, 128 * jb:128 * (jb + 1)]

            # One persistent staging buffer for the whole per-core output.
            # No slot recycling -> no release waits, so every DVE copy carries
            # only its PE wait (walrus allows a single sync wait per DVE /
            # matmul instruction).
            stage = stagep.tile([128, NBLK, G, T], f32)

            # local row = 128*p + 8*s + g: per-partition output is linear in
            # HBM, so writeback DMAs are long contiguous bursts per partition.
            out_lin = out_d.rearrange("(p s g) t -> p s g t", p=128, s=NBLK, g=G)

            # Alternate PSUM->SBUF copies between DVE and ACT per DMA group so
            # neither engine's copy throughput gates the pipeline, while each
            # output DMA still waits on a single engine's semaphore.
            grp_of_blk = []
            for gi, n in enumerate(DMA_GROUPS):
                grp_of_blk += [gi] * n
            bounds = {sum(DMA_GROUPS[:gi + 1]) - 1 for gi in range(len(DMA_GROUPS))}

            s0 = 0
            for s in range(NBLK):
                use_dve = grp_of_blk[s] % 2 == 0
                for dj in range(2):
                    j = 2 * s + dj
                    if j == A_QUADS // 2:
                        # Tiny claim matmul: absorbs the B-chunk DMA wait on
                        # the PE clock so later B matmuls carry only their
                        # psum-release wait. Placed mid-A so PE never stalls.
                        cps = clmp.tile([128, 8], f32)
                        nc.tensor.matmul(
                            cps[:1, :1],
                            vtsB[:1, :1],
                            vtsB[:1, 1:2],
                            start=True,
                            stop=True,
                            tile_position=(0, 0),
                        )
                    for q in range(4):
                        ps = psump.tile([128, T], f32)
                        nc.tensor.matmul(
                            ps[:],
                            lhsT(j, q),
                            hrep[32 * q:32 * q + 32, :],
                            start=True,
                            stop=True,
                            tile_position=(32 * q, 0),
                        )
                        g = 4 * dj + q
                        if use_dve:
                            nc.vector.tensor_copy(stage[:, s, g, :], ps[:])
                        else:
                            nc.scalar.copy(stage[:, s, g, :], ps[:])
                if s in bounds:
                    # All output groups on the single SP HWDGE ring: splitting
                    # across SWDGE (81.2us) or the ACT HWDGE ring (80.5us)
                    # measured strictly worse than one saturated ring (74.0us)
                    # — interleaved rings fragment the HBM write stream.
                    nc.sync.dma_start(
                        out_lin[:, s0:s + 1, :, :], stage[:, s0:s + 1, :, :]
                    )
                    s0 = s + 1

    _nc_cache.append(nc)
    return nc


def _run(in_maps, trace=False):
    from concourse.bass_utils import run_bass_kernel_spmd

    nc = _build_bass()
    return run_bass_kernel_spmd(nc, in_maps, list(range(NCORES)), trace=trace)


def kernel(x, c=None, sigma2=None, scale=None, _trace=False):
    vts, hc = _pack_inputs(x, c, sigma2, scale)
    acols = 128 * A_QUADS
    in_maps = [
        {
            "va": np.ascontiguousarray(
                np.concatenate([vts[i][:, :acols], hc], axis=1)
            ),
            "vb": np.ascontiguousarray(vts[i][:, acols:]),
        }
        for i in range(NCORES)
    ]
    res = _run(in_maps, trace=_trace)
    out = np.concatenate([res.results[i]["out"] for i in range(NCORES)], axis=0)
    out = out.reshape(BATCH, DOF, T)
    if _trace:
        return out, res
    return out



# revision 3
# speedup vs baseline: 1.1235x; 1.1235x over previous
"""Trainium2 Bass kernel for the DMP-rollout Net (nn_Net_60567628808344).

Math
----
The reference integrates, per row r of p = (x*scale).reshape(-1, 27):
    y0 = p[:,0], goal = p[:,1], w = p[:,2:]
    cx_j = (1 - A_X*DT/TAU)^j                     (data independent)
    psi_j = exp(-0.5 (cx_j - c)^2 / sigma2)       (data independent)
    state update is LINEAR:  s_j = M s_{j-1} + [0; k*(Az*Bz*goal + F_j)]
    with F_j = (w @ psi_j) * cx_j * (goal-y0) / sum(psi_j)
So the whole 301-step rollout collapses to a closed form
    out[r, i] = A[i]*y0_r + B[i]*goal_r + (goal_r - y0_r) * (w_r @ H[i, :])
with constant A,B (301,), H (301, 25) precomputed in float64 on host.

Device work per core (8-way batch data-parallel) is then a single
(16384 x 27) @ (27 x 301) matmul + the 19.7 MB/core output writeback,
which is HBM-write roofline bound (~430 GB/s across the 16 DMA engines).

Inputs are bf16 (tolerance is 2e-2; bf16 keeps rel err ~1e-3) so the PE
produces at ~2x the DMA drain rate instead of pacing it, and the input
load halves. Output staging is tile-major so writeback DMA groups ramp
up (tiny first group -> output bytes start flowing ~13us instead of
~18us).

Layout: per-core rows are packed 4 tiles at a time into the partition
dim (4 groups of 32 partitions, K padded 27->32 with zeros) so each
quad issues 4 row-group-packed concurrent matmuls (tile_position
(32q, 0) auto-derived from base partitions).
"""

import numpy as np
import ml_dtypes

# DMP hyperparameters fixed by Net.__init__ (hardcoded per problem spec)
N = 25
DOF = 2
DT = 0.01
TAU = 3.0
A_X = 2.0
A_Z = 48.0
B_Z = A_Z / 4.0
T = 301                    # time steps
BATCH = 65536
PARAM_DIM = DOF * (N + 2)  # 54
NCORES = 8

ROWS = BATCH * DOF         # 131072 (B*DOF rows)
RPC = ROWS // NCORES       # 16384 rows per core
TILES = RPC // 128         # 128 tiles of 128 rows per core
QUAD_COLS = RPC // 4       # 4096: vt free dim (4 tiles packed in partition dim)
# Output DMA groups in tiles (128 total). Small head so writeback bytes
# start flowing as early as possible; large middle amortizes descriptor
# fetch. All 8 sync-HWDGE sem lanes go to output (va rides the scalar
# HWDGE ring, vb the gpsimd SWDGE).
DMA_GROUPS = (2, 6, 16, 24, 28, 28, 24)
assert sum(DMA_GROUPS) == TILES and len(DMA_GROUPS) <= 7

A_QUADS = 8                # quads in the head input chunk


# ----------------------------------------------------------------------------
# Host-side constant build (exact, float64)
# ----------------------------------------------------------------------------
_const_cache = {}


def _build_constants(c=None, sigma2=None):
    """Return hc (128, 301) bf16: rows 32q+p hold
    p==0 -> A, p==1 -> B, p==2+n -> H[:, n], rows 27..31 of each group zero."""
    if c is None:
        c = np.exp(-A_X * np.linspace(0.0, 1.0, N))
    if sigma2 is None:
        sigma2 = (N ** 1.5) / c / A_X
    c = np.asarray(c, np.float64)
    sigma2 = np.asarray(sigma2, np.float64)
    key = (c.tobytes(), sigma2.tobytes())
    if key in _const_cache:
        return _const_cache[key]

    k = DT / TAU
    M = np.array([[1.0, k], [-A_Z * B_Z * k, 1.0 - A_Z * k]])
    P = np.zeros(T + 1)
    Q = np.zeros(T + 1)
    Mn = np.eye(2)
    for n in range(T + 1):
        P[n] = Mn[0, 0]
        Q[n] = Mn[0, 1]
        Mn = Mn @ M

    decay = 1.0 - A_X * DT / TAU
    cx = decay ** np.arange(1, T + 1)                        # cx_1..cx_T
    psi = np.exp(-0.5 * (cx[:, None] - c[None, :]) ** 2 / sigma2[None, :])
    g = psi * (cx / psi.sum(1))[:, None]                     # (T, N)

    A = P[1:T + 1]
    B = k * A_Z * B_Z * np.cumsum(Q[0:T])
    # H[i] = k * sum_{m<=i} Q[i-m] g[m]  -- lower-triangular Toeplitz matvec
    ii = np.arange(T)[:, None]
    mm = np.arange(T)[None, :]
    L = np.where(ii >= mm, Q[np.clip(ii - mm, 0, T)], 0.0)   # (T, T)
    H = k * (L @ g)                                          # (T, N)

    hfull = np.zeros((32, T), np.float32)
    hfull[0] = A.astype(np.float32)
    hfull[1] = B.astype(np.float32)
    hfull[2:2 + N] = H.T.astype(np.float32)
    hc = np.tile(hfull, (4, 1)).astype(ml_dtypes.bfloat16)   # (128, T)
    _const_cache[key] = hc
    return hc


def _pack_inputs(x, c, sigma2, scale):
    """Build per-core vt arrays (128, 4096) bf16 + shared hc (128, 301)."""
    x = np.asarray(x, np.float32)
    if scale is None:
        scale = np.ones(PARAM_DIM, np.float32)
    p = (x * np.asarray(scale, np.float32)).reshape(ROWS, N + 2)
    y0 = p[:, 0]
    goal = p[:, 1]
    u = goal - y0
    v = np.empty((ROWS, N + 2), np.float32)
    v[:, 0] = y0
    v[:, 1] = goal
    v[:, 2:] = p[:, 2:] * u[:, None]
    v = v.astype(ml_dtypes.bfloat16)

    hc = _build_constants(c, sigma2)

    vts = []
    for i in range(NCORES):
        vc = v[RPC * i:RPC * (i + 1)]                 # (16384, 27)
        # Tile t=4j+q, lhsT free index f computes local row 128*f + t, so
        # each out-partition owns a contiguous run of HBM rows (linear
        # writeback).  row = 128*f + 4*j + q -> [f, j, q, p] -> [q, p, j, f]
        v4 = vc.reshape(128, TILES // 4, 4, N + 2).transpose(2, 3, 1, 0)
        vp = np.zeros((4, 32, TILES // 4, 128), ml_dtypes.bfloat16)
        vp[:, :N + 2] = v4
        vts.append(np.ascontiguousarray(vp.reshape(128, QUAD_COLS)))
    return vts, hc


# ----------------------------------------------------------------------------
# Bass kernel
# ----------------------------------------------------------------------------
_nc_cache = []


def _build_bass():
    if _nc_cache:
        return _nc_cache[0]
    import concourse.bass as bass
    import concourse.mybir as mybir
    from concourse import tile
    import bass_rust
    from concourse.vector_clock import ScopedClock

    class SplitDrainTileContext(tile.TileContext):
        """This walrus build allows a single sync wait per instruction, but
        TileContext's kernel-tail drain carries one wait per live sem lane.
        Split the extras onto standalone single-wait SP nops (same stream, so
        all waits still complete before the barrier + sem clearing)."""

        def _drain_and_barrier(self, tick_clock, wait_clock):
            nc = self.nc
            drain_inst = nc.sync.drain()
            wait_clock.add_sem_waits(
                drain_inst.ins, ScopedClock({None: tick_clock.global_clock})
            )
            si = drain_inst.ins.sync_info
            waits = list(si.on_wait) if si is not None else []
            if len(waits) > 1:
                drain_inst.ins.sync_info = bass_rust.SyncInfo(
                    on_wait=[waits[0]], on_update=list(si.on_update)
                )
                for w in waits[1:]:
                    n = nc.sync.nop(nofuse=True)
                    n.ins.sync_info = bass_rust.SyncInfo(
                        on_wait=[w], on_update=[]
                    )
            nc.all_engine_barrier()
            assert self.sems is not None
            popped = nc._tile_sem_poison_stack.pop()
            assert popped is self._sem_poison
            nc.clear_and_free_semaphores(list(self.sems.allocated().values()))
            nc.all_engine_barrier()

    f32 = mybir.dt.float32
    fmm = mybir.dt.bfloat16
    nc = bass.Bass()
    # Input split: a small head chunk (first A_QUADS quads + the 301 constant
    # columns) on the scalar HWDGE ring so compute starts early, the rest on
    # SWDGE in parallel. Single tensors per chunk keep every matmul at one
    # sync wait (walrus allows a single S3_LW wait slot per self-loading
    # matmul).
    va_d = nc.dram_tensor("va", [128, 128 * A_QUADS + T], fmm, kind="ExternalInput")
    vb_d = nc.dram_tensor(
        "vb", [128, QUAD_COLS - 128 * A_QUADS], fmm, kind="ExternalInput"
    )
    out_d = nc.dram_tensor("out", [RPC, T], f32, kind="ExternalOutput")

    with SplitDrainTileContext(nc) as tc:
        with (
            tc.tile_pool(name="vtp", bufs=1) as vtp,
            tc.tile_pool(name="stage", bufs=1) as stagep,
            tc.tile_pool(name="psum", bufs=7, space="PSUM") as psump,
            tc.tile_pool(name="clm", bufs=1, space="PSUM") as clmp,
        ):
            vtsA = vtp.tile([128, 128 * A_QUADS + T], fmm, tag="vtsA")
            vtsB = vtp.tile([128, QUAD_COLS - 128 * A_QUADS], fmm, tag="vtsB")
            nc.scalar.dma_start(vtsA[:], va_d[:])
            nc.gpsimd.dma_start(vtsB[:], vb_d[:])
            hrep = vtsA[:, 128 * A_QUADS:128 * A_QUADS + T]

            def lhsT(j, q):
                if j < A_QUADS:
                    return vtsA[32 * q:32 * q + 32, 128 * j:128 * (j + 1)]
                jb = j - A_QUADS
                return vtsB[32 * q:32 * q + 32, 128 * jb:128 * (jb + 1)]

            # One persistent staging buffer for the whole per-core output.
            # No slot recycling -> no release waits, so every DVE copy carries
            # only its PE wait (walrus allows a single sync wait per DVE /
            # matmul instruction).
            stage = stagep.tile([128, TILES, T], f32)

            # local row = 128*p + tile: per-partition output is linear in
            # HBM, so writeback DMAs are long contiguous bursts per partition.
            out_lin = out_d.rearrange("(p r) t -> p r t", p=128, r=TILES)

            # Alternate PSUM->SBUF copies between DVE and ACT per DMA group so
            # neither engine's copy throughput gates the pipeline, while each
            # output DMA still waits on a single engine's semaphore.
            grp_of_tile = []
            for gi, n in enumerate(DMA_GROUPS):
                grp_of_tile += [gi] * n
            bounds = {sum(DMA_GROUPS[:gi + 1]) - 1 for gi in range(len(DMA_GROUPS))}

            t0 = 0
            for j in range(TILES // 4):
                if j == A_QUADS // 2:
                    # Tiny claim matmul: absorbs the B-chunk DMA wait on
                    # the PE clock so later B matmuls carry only their
                    # psum-release wait. Placed mid-A so PE never stalls.
                    cps = clmp.tile([128, 8], f32)
                    nc.tensor.matmul(
                        cps[:1, :1],
                        vtsB[:1, :1],
                        vtsB[:1, 1:2],
                        start=True,
                        stop=True,
                        tile_position=(0, 0),
                    )
                for q in range(4):
                    tile_i = 4 * j + q
                    ps = psump.tile([128, T], f32)
                    nc.tensor.matmul(
                        ps[:],
                        lhsT(j, q),
                        hrep[32 * q:32 * q + 32, :],
                        start=True,
                        stop=True,
                        tile_position=(32 * q, 0),
                    )
                    if grp_of_tile[tile_i] % 2 == 0:
                        nc.vector.tensor_copy(stage[:, tile_i, :], ps[:])
                    else:
                        nc.scalar.copy(stage[:, tile_i, :], ps[:])
                    if tile_i in bounds:
                        # All output groups on the single sync HWDGE ring:
                        # splitting across SWDGE or the ACT HWDGE ring
                        # measured strictly worse — interleaved rings
                        # fragment the HBM write stream.
                        nc.sync.dma_start(
                            out_lin[:, t0:tile_i + 1, :], stage[:, t0:tile_i + 1, :]
                        )
                        t0 = tile_i + 1

    _nc_cache.append(nc)
    return nc


def _run(in_maps, trace=False):
    from concourse.bass_utils import run_bass_kernel_spmd

    nc = _build_bass()
    return run_bass_kernel_spmd(nc, in_maps, list(range(NCORES)), trace=trace)


def kernel(x, c=None, sigma2=None, scale=None, _trace=False):
    vts, hc = _pack_inputs(x, c, sigma2, scale)
    acols = 128 * A_QUADS
    in_maps = [
        {
            "va": np.ascontiguousarray(
                np.concatenate([vts[i][:, :acols], hc], axis=1)
            ),
            "vb": np.ascontiguousarray(vts[i][:, acols:]),
        }
        for i in range(NCORES)
    ]
    res = _run(in_maps, trace=_trace)
    out = np.concatenate([res.results[i]["out"] for i in range(NCORES)], axis=0)
    out = out.reshape(BATCH, DOF, T)
    if _trace:
        return out, res
    return out
